# revision 42
# baseline (speedup 1.0000x reference)
"""GCN dialogue manager kernel for 8 trn2 NeuronCores.

Math (reference):
    h   = x @ W_gcn
    deg = in_deg(dst) + 1   (self loops)
    dinv = rsqrt(deg)
    agg[d] = sum_{e:(s->d)} dinv[s]*dinv[d]*h[s] + dinv[d]^2*h[d] + b_gcn
    out = agg @ W_act + b_act

Kernel strategy (dst-sharded, edges partitioned by destination):
    out[d] = (sum_{slots of d} norm[e]*x[s]) @ (W_gcn@W_act) + dinv[d]^2*x[d]@W
             + (b_gcn@W_act + b_act),   norm[e] = rsqrt(deg[s]*deg[d])
  - 8 cores each own 6250 destination nodes (49 blocks of 128).
  - Real edges (NO self loops) are bucketed by (core, dst-block, src-half) on
    the host (integer metadata only) into a static per-group tile schedule
    (ntile[group] = cross-core max, data-driven).
  - On device: dma_gather fetches fp16 x rows (256B) per slot from one of two
    half-tables (int16 index limit, <=1024 idx per call = SWDGE ring cap,
    4 SWDGE queues for parallel descriptor generation).
  - Routing: sel built chunk-wide in [P, dst, col] layout so every DVE
    operand has a stride-1 last dim (2x perf mode): sel = is_equal(dstloc,
    iota) * norm, two big fp16 tensor_tensor ops; TensorE accumulates
    accT[f, d] += xg_col^T @ sel[:, :, col] in fp32 PSUM (transposed
    orientation: no per-block transpose needed at flush).
  - Self loops: per-block matmul accT += xself_blk^T @ diag(1/deg), with
    xself = the core's own x rows staged contiguously (no descriptors).
  - Flush per block: copy accT PSUM->SBUF, matmul with lhsT = fused
    W = W_gcn@W_act giving out^T[a, d]; fused bias added via the scalar
    activation's per-partition bias; out^T accumulates in SBUF and is
    written once at the end (host transposes back).
"""

import os
import sys

for _p in ("/opt/trn_rl_repo",):
    if _p not in sys.path and os.path.isdir(_p):
        sys.path.insert(0, _p)

import numpy as np

# ---- problem constants (hardcoded per spec) ----
N, E, F, HID, A = 50000, 600000, 128, 128, 64
P = 128                      # partitions
NCORE = 8
DST_PER_CORE = 6250          # N / 8
NBLK = 49                    # ceil(6250/128) dst blocks per core
OUT_ROWS = NBLK * P          # 6272 padded out rows per core
HALF = 25152                 # nodes [0,HALF) in table A, [HALF,...) in table B
XROWS = 25216                # rows per half table (HALF + 64 zero pad rows)
ZROW_A = 25152               # a zero row in table A (explicit pad row)
ZROW_B = 25024               # zero row in table B (> N-HALF real rows)
MAXTPG = 8                   # hard cap: 1024 slots per gather call (ring cap)
CHUNK = 4                    # dst blocks per compute chunk
_CHUNKS = [(c * CHUNK, min(CHUNK, NBLK - c * CHUNK)) for c in range((NBLK + CHUNK - 1) // CHUNK)]

_prog_cache = {}


def _build_program(ntile, gmax):
    """Build the Bass program shared by all 8 cores.

    ntile: tuple of 98 ints — tiles (of 128 slots) per (block, half) group,
    group index g = blk*2 + half.
    gmax: tuple of 98 ints — cross-core max REAL slot count per group,
    rounded up to 16 (the rest of each gather call is -1 padding, which the
    SWDGE skips: no descriptors, no bytes)."""
    key = (tuple(ntile), tuple(gmax))
    if key in _prog_cache:
        return _prog_cache[key]

    import concourse.bacc as bacc
    import concourse.mybir as mybir
    import concourse.tile as tile
    from concourse.masks import make_identity

    f32 = mybir.dt.float32
    f16 = mybir.dt.float16
    i32 = mybir.dt.int32
    i16 = mybir.dt.int16
    Alu = mybir.AluOpType
    Act = mybir.ActivationFunctionType

    col_start = np.concatenate([[0], np.cumsum(ntile)]).astype(int)
    tot_col = int(col_start[-1])
    tot_slot = tot_col * P

    nc = bacc.Bacc(None, num_swdge_queues=4)

    xa = nc.dram_tensor("xa", [XROWS, F], f16, kind="ExternalInput")
    xb = nc.dram_tensor("xb", [XROWS, F], f16, kind="ExternalInput")
    xself = nc.dram_tensor("xself", [P, NBLK * F], f16, kind="ExternalInput")
    idxs = nc.dram_tensor("idxs", [P, tot_slot // 16], i16, kind="ExternalInput")
    dstloc = nc.dram_tensor("dstloc", [P, tot_col], f32, kind="ExternalInput")
    degprod = nc.dram_tensor("degprod", [P, tot_col], f32, kind="ExternalInput")
    degdst = nc.dram_tensor("degdst", [P, NBLK], f32, kind="ExternalInput")
    wgT = nc.dram_tensor("wgT", [HID, F], f32, kind="ExternalInput")
    wact = nc.dram_tensor("wact", [HID, A], f32, kind="ExternalInput")
    bgcn = nc.dram_tensor("bgcn", [HID, 1], f32, kind="ExternalInput")
    bact = nc.dram_tensor("bact", [A, 1], f32, kind="ExternalInput")
    out = nc.dram_tensor("out", [A, OUT_ROWS], f32, kind="ExternalOutput")

    with tile.TileContext(nc) as tc:
        with (
            tc.tile_pool(name="const", bufs=1) as cpool,
            tc.tile_pool(name="cpsum", bufs=1, space="PSUM") as cpsum,
            tc.tile_pool(name="xg", bufs=4) as xgpool,
            tc.tile_pool(name="sel", bufs=3) as spool,
            tc.tile_pool(name="acc", bufs=3, space="PSUM") as accpool,
            tc.tile_pool(name="outp", bufs=2, space="PSUM") as outppool,
            tc.tile_pool(name="flush", bufs=3) as fpool,
        ):
            # ---- constants / prologue ----
            idx_sb = cpool.tile([P, tot_slot // 16], i16)
            nc.sync.dma_start(out=idx_sb[:], in_=idxs[:])

            dstloc_sb = cpool.tile([P, tot_col], f32)
            nc.sync.dma_start(out=dstloc_sb[:], in_=dstloc[:])
            dstloc_h = cpool.tile([P, tot_col], f16)
            nc.vector.tensor_copy(out=dstloc_h[:], in_=dstloc_sb[:])

            # norm = rsqrt(deg[src]*deg[dst]) per slot (fp16 for 2x DVE mode)
            norm_sb = cpool.tile([P, tot_col], f32)
            nc.sync.dma_start(out=norm_sb[:], in_=degprod[:])
            nc.vector.reciprocal(out=norm_sb[:], in_=norm_sb[:])
            nc.scalar.activation(norm_sb[:], norm_sb[:], Act.Sqrt)
            norm_h = cpool.tile([P, tot_col], f16)
            nc.vector.tensor_copy(out=norm_h[:], in_=norm_sb[:])

            # dinv^2 = 1/deg for the self-loop diagonal
            dinvsq = cpool.tile([P, NBLK], f32)
            nc.sync.dma_start(out=dinvsq[:], in_=degdst[:])
            nc.vector.reciprocal(out=dinvsq[:], in_=dinvsq[:])

            ident = cpool.tile([P, P], f32)
            make_identity(nc, ident[:])
            ident_h = cpool.tile([P, P], f16)
            nc.vector.tensor_copy(out=ident_h[:], in_=ident[:])

            iota_i = cpool.tile([P, P], i32)
            nc.gpsimd.iota(iota_i[:], pattern=[[1, P]], base=0, channel_multiplier=0)
            iota_h = cpool.tile([P, P], f16)
            nc.vector.tensor_copy(out=iota_h[:], in_=iota_i[:])
            # iota materialized along the middle (dst) dim: iota_mid[p,d,c] = d
            chmax = max(int(col_start[(b0 + ncb) * 2] - col_start[b0 * 2])
                        for (b0, ncb) in _CHUNKS)
            iota_mid = cpool.tile([P, P, chmax], f16)
            nc.vector.tensor_copy(
                out=iota_mid[:],
                in_=iota_h[:].unsqueeze(2).broadcast_to([P, P, chmax]),
            )

            # per-block self-loop diagonal: diag[q, b, d] = (q==d) * dinvsq[q, b]
            diag = cpool.tile([P, NBLK, P], f16)
            nc.vector.tensor_tensor(
                out=diag[:],
                in0=ident_h[:].unsqueeze(1).broadcast_to([P, NBLK, P]),
                in1=dinvsq[:].unsqueeze(2).broadcast_to([P, NBLK, P]),
                op=Alu.mult,
            )

            xself_sb = cpool.tile([P, NBLK * F], f16)
            nc.sync.dma_start(out=xself_sb[:], in_=xself[:])

            wgT_sb = cpool.tile([HID, F], f32)
            nc.sync.dma_start(out=wgT_sb[:], in_=wgT[:])
            wact_sb = cpool.tile([HID, A], f32)
            nc.sync.dma_start(out=wact_sb[:], in_=wact[:])
            wf_ps = cpsum.tile([F, A], f32, space="PSUM", tag="cps")
            nc.tensor.matmul(wf_ps[:], lhsT=wgT_sb[:], rhs=wact_sb[:], start=True, stop=True)
            # fp16 so the steady-state matmul stream is all-fp16 (keeps FWL on)
            wf_h = cpool.tile([F, A], f16)
            nc.vector.tensor_copy(out=wf_h[:], in_=wf_ps[:])

            # fused bias, transposed: cbT[a] = sum_h bgcn[h]*W_act[h,a] + bact[a]
            bgcn_sb = cpool.tile([HID, 1], f32)
            nc.sync.dma_start(out=bgcn_sb[:], in_=bgcn[:])
            bact_sb = cpool.tile([A, 1], f32)
            nc.sync.dma_start(out=bact_sb[:], in_=bact[:])
            cb_ps = cpsum.tile([A, 1], f32, space="PSUM", tag="cps")
            nc.tensor.matmul(cb_ps[:], lhsT=wact_sb[:], rhs=bgcn_sb[:], start=True, stop=True)
            cb_sb = cpool.tile([A, 1], f32)
            nc.vector.tensor_copy(out=cb_sb[:], in_=cb_ps[:])
            nc.vector.tensor_tensor(out=cb_sb[:], in0=cb_sb[:], in1=bact_sb[:], op=Alu.add)

            # transposed output accumulator, written to DRAM once at the end
            out_all = cpool.tile([A, OUT_ROWS], f32)

            num_regs = {int(v): nc.gpsimd.to_reg(int(v))
                        for v in sorted(set(int(t) for t in gmax))}

            # PE HAM warm-up: ~40 back-to-back matmuls (~9us cold) trip the
            # activity monitor to 8/8 clock (2.4GHz) before the real stream;
            # overlaps the prologue input DMAs.
            warm_ps = cpsum.tile([P, P], f32, space="PSUM", tag="warm")
            for _ in range(40):
                nc.tensor.matmul(warm_ps[:], lhsT=ident_h[:], rhs=ident_h[:],
                                 start=True, stop=True)

            # ---- main loop over chunks of dst blocks ----
            qn = 0
            for ci, (b0, ncb) in enumerate(_CHUNKS):
                c0 = int(col_start[b0 * 2])
                ncols = int(col_start[(b0 + ncb) * 2] - c0)
                xg = xgpool.tile([P, ncols, F], f16, tag="xg")
                for i in range(ncb):
                    for h, tab in ((0, xa), (1, xb)):
                        g = (b0 + i) * 2 + h
                        nt = int(ntile[g])
                        num = nt * P
                        crel = int(col_start[g]) - c0
                        s0 = int(col_start[g]) * P
                        nc.gpsimd.dma_gather(
                            xg[:, crel: crel + nt, :],
                            tab[:],
                            idx_sb[:, s0 // 16: (s0 + num) // 16],
                            num,
                            num_regs[int(gmax[g])],
                            F,
                            queue_num=qn % 4,
                        )
                        qn += 1
                # norm-scaled one-hot, [P, dst, col] layout: every operand has
                # a stride-1 last dim -> DVE 2x perf mode on both big ops
                # sel[q, d, col] = (dstloc[q, col] == d) * norm[q, col]
                sel = spool.tile([P, P, ncols], f16, tag="sel")
                nc.vector.tensor_tensor(
                    out=sel[:],
                    in0=dstloc_h[:, c0:c0 + ncols].unsqueeze(1).broadcast_to([P, P, ncols]),
                    in1=iota_mid[:, :, :ncols],
                    op=Alu.is_equal,
                )
                nc.vector.tensor_tensor(
                    out=sel[:],
                    in0=sel[:],
                    in1=norm_h[:, c0:c0 + ncols].unsqueeze(1).broadcast_to([P, P, ncols]),
                    op=Alu.mult,
                )
                for i in range(ncb):
                    b = b0 + i
                    gA, gB = b * 2, b * 2 + 1
                    # accT[f, d] accumulated transposed: no flush transpose
                    acc = accpool.tile([P, P], f32, space="PSUM", tag="acc")
                    nc.tensor.matmul(
                        acc[:],
                        lhsT=xself_sb[:, b * F:(b + 1) * F],
                        rhs=diag[:, b, :],
                        start=True,
                        stop=False,
                    )
                    cols = list(range(int(col_start[gA]) - c0, int(col_start[gB + 1]) - c0))
                    for j, col in enumerate(cols):
                        nc.tensor.matmul(
                            acc[:],
                            lhsT=xg[:, col, :],
                            rhs=sel[:, :, col],
                            start=False,
                            stop=(j == len(cols) - 1),
                        )
                    # flush block b: out^T[a, d] = wf^T @ accT + cbT
                    accTs = fpool.tile([P, P], f16, tag="accTs")
                    nc.scalar.activation(accTs[:], acc[:], Act.Copy)
                    outp = outppool.tile([A, P], f32, space="PSUM", tag="outp")
                    nc.tensor.matmul(outp[:], lhsT=wf_h[:], rhs=accTs[:], start=True, stop=True)
                    nc.scalar.activation(
                        out_all[:, b * P:(b + 1) * P], outp[:], Act.Identity,
                        bias=cb_sb[:, 0:1],
                    )
            nc.sync.dma_start(out=out[:], in_=out_all[:])

    nc.compile()
    _prog_cache[key] = nc
    return nc


def _preprocess(x, edge_index):
    """Host-side sharding: bucket edges by (core, dst block, src half) and
    build the static padded slot arrays. Integer/layout work only."""
    src = np.asarray(edge_index[0], dtype=np.int64)
    dst = np.asarray(edge_index[1], dtype=np.int64)

    in_deg = np.bincount(dst, minlength=N).astype(np.int64)
    deg_tot = in_deg + 1  # self loop

    core = dst // DST_PER_CORE
    loc = dst % DST_PER_CORE
    blk = loc >> 7
    dloc = loc & 127
    half = (src >= HALF).astype(np.int64)
    rowid = src - HALF * half
    dprod = deg_tot[src] * deg_tot[dst]

    # group = (core, blk, half); position within group via stable sort
    g = (core * NBLK + blk) * 2 + half
    order = np.argsort(g, kind="stable")
    g_sorted = g[order]
    cnt = np.bincount(g_sorted, minlength=NCORE * NBLK * 2)
    # static tile schedule: cross-core max per (blk, half) group
    cnt2 = cnt.reshape(NCORE, NBLK * 2)
    ntile = np.maximum(1, -(-cnt2.max(axis=0) // P))  # [98]
    if ntile.max() > MAXTPG:
        raise RuntimeError(f"group needs {ntile.max()} tiles > {MAXTPG}")
    col_start = np.concatenate([[0], np.cumsum(ntile)]).astype(np.int64)
    tot_col = int(col_start[-1])
    tot_slot = tot_col * P

    starts = np.zeros_like(cnt)
    starts[1:] = np.cumsum(cnt)[:-1]
    pos_in_group = np.arange(len(order)) - starts[g_sorted]

    blk_s = blk[order]
    half_s = half[order]
    g2 = blk_s * 2 + half_s
    col = col_start[g2] + (pos_in_group >> 7)
    p = pos_in_group & 127
    flat = col * P + p  # slot id within core

    core_s = core[order]
    rowid_s = rowid[order]
    dloc_s = dloc[order]
    dprod_s = dprod[order]

    # per-core output arrays (padded defaults; pad slots gather a zero row —
    # num_idxs_reg must equal the exact non-negative idx count, so variable
    # per-core counts would need per-core registers, which serialize the Q7s)
    idx_arr = np.empty((NCORE, tot_slot), dtype=np.int16)
    colg = np.repeat(np.arange(NBLK * 2), ntile)  # group of each column
    pad_idx = np.where(colg % 2 == 1, ZROW_B, ZROW_A).astype(np.int16)
    idx_arr[:] = np.repeat(pad_idx, P)[None, :]
    dst_arr = np.full((NCORE, tot_slot), -1.0, dtype=np.float32)
    dpr_arr = np.ones((NCORE, tot_slot), dtype=np.float32)
    gmax = (ntile * P).astype(np.int64)

    lin = core_s * tot_slot + flat
    idx_arr.reshape(-1)[lin] = rowid_s.astype(np.int16)
    dst_arr.reshape(-1)[lin] = dloc_s.astype(np.float32)
    dpr_arr.reshape(-1)[lin] = dprod_s.astype(np.float32)

    # idxs: 16-partition wrap replicated 8x -> [128, tot_slot//16]
    idx_wrap = idx_arr.reshape(NCORE, tot_slot // 16, 16).transpose(0, 2, 1)
    idx_rep = np.tile(idx_wrap, (1, 8, 1)).copy()



    # dstloc/degprod: [128, tot_col] with value at [p, col]
    dst_pc = dst_arr.reshape(NCORE, tot_col, P).transpose(0, 2, 1).copy()
    dpr_pc = dpr_arr.reshape(NCORE, tot_col, P).transpose(0, 2, 1).copy()

    # degdst: [NCORE, 128, NBLK]
    degdst = np.ones((NCORE, P, NBLK), dtype=np.float32)
    node = np.arange(N, dtype=np.int64)
    nc_ = node // DST_PER_CORE
    nl = node % DST_PER_CORE
    degdst[nc_, nl & 127, nl >> 7] = deg_tot.astype(np.float32)

    # x half tables (fp16, zero padded)
    x16 = x.astype(np.float16)
    xa = np.zeros((XROWS, F), dtype=np.float16)
    xa[:HALF] = x16[:HALF]
    xb = np.zeros((XROWS, F), dtype=np.float16)
    xb[: N - HALF] = x16[HALF:]

    # per-core own x rows, packed [128, NBLK*F]: partition p holds rows
    # {p, 128+p, ...} of the core's shard (for the self-loop diagonal matmul)
    xself = np.zeros((NCORE, P, NBLK * F), dtype=np.float16)
    for c in range(NCORE):
        shard = np.zeros((OUT_ROWS, F), dtype=np.float16)
        shard[:DST_PER_CORE] = x16[c * DST_PER_CORE:(c + 1) * DST_PER_CORE]
        xself[c] = shard.reshape(NBLK, P, F).transpose(1, 0, 2).reshape(P, NBLK * F)

    return ntile, gmax, xa, xb, xself, idx_rep, dst_pc, dpr_pc, degdst


def kernel(x, edge_index, W_gcn, b_gcn, W_act, b_act):
    from concourse.bass_utils import run_bass_kernel_spmd

    x = np.ascontiguousarray(np.asarray(x, dtype=np.float32))
    ntile, gmax, xa, xb, xself, idx_rep, dst_pc, dpr_pc, degdst = _preprocess(x, edge_index)

    wgT = np.ascontiguousarray(np.asarray(W_gcn, dtype=np.float32).T)
    wact = np.ascontiguousarray(np.asarray(W_act, dtype=np.float32))
    bg = np.ascontiguousarray(np.asarray(b_gcn, dtype=np.float32).reshape(HID, 1))
    ba = np.ascontiguousarray(np.asarray(b_act, dtype=np.float32).reshape(A, 1))

    nc = _build_program(tuple(int(v) for v in ntile), tuple(int(v) for v in gmax))
    in_maps = [
        {
            "xa": xa,
            "xb": xb,
            "xself": xself[c],
            "idxs": idx_rep[c],
            "dstloc": dst_pc[c],
            "degprod": dpr_pc[c],
            "degdst": degdst[c],
            "wgT": wgT,
            "wact": wact,
            "bgcn": bg,
            "bact": ba,
        }
        for c in range(NCORE)
    ]
    trace = bool(os.environ.get("GCN_TRACE"))
    res = run_bass_kernel_spmd(nc, in_maps, core_ids=list(range(NCORE)), trace=trace)
    kernel.last_results = res

    out = np.concatenate(
        [res.results[c]["out"].T[:DST_PER_CORE] for c in range(NCORE)], axis=0
    )
    return np.ascontiguousarray(out, dtype=np.float32)


# revision 46
# speedup vs baseline: 1.1699x; 1.1699x over previous
"""GCN dialogue manager kernel for 8 trn2 NeuronCores.

Math (reference):
    h   = x @ W_gcn
    deg = in_deg(dst) + 1   (self loops)
    dinv = rsqrt(deg)
    agg[d] = sum_{e:(s->d)} dinv[s]*dinv[d]*h[s] + dinv[d]^2*h[d] + b_gcn
    out = agg @ W_act + b_act

Kernel strategy (dst-sharded, edges partitioned by destination):
    out[d] = (sum_{slots of d} norm[e]*x[s]) @ (W_gcn@W_act) + dinv[d]^2*x[d]@W
             + (b_gcn@W_act + b_act),   norm[e] = rsqrt(deg[s]*deg[d])
  - 8 cores each own 6250 destination nodes (49 blocks of 128).
  - Real edges (NO self loops) are bucketed by (core, dst-block, src-half) on
    the host (integer metadata only) into a static per-group tile schedule
    (ntile[group] = cross-core max, data-driven).
  - On device: dma_gather fetches fp16 x rows (256B) per slot from one of two
    half-tables (int16 index limit, <=1024 idx per call = SWDGE ring cap,
    4 SWDGE queues for parallel descriptor generation).
  - Routing: sel built chunk-wide in [P, dst, col] layout so every DVE
    operand has a stride-1 last dim (2x perf mode): sel = is_equal(dstloc,
    iota) * norm, two big fp16 tensor_tensor ops; TensorE accumulates
    accT[f, d] += xg_col^T @ sel[:, :, col] in fp32 PSUM (transposed
    orientation: no per-block transpose needed at flush).
  - Self loops: per-block matmul accT += xself_blk^T @ diag(1/deg), with
    xself = the core's own x rows staged contiguously (no descriptors).
  - Flush per block: copy accT PSUM->SBUF, matmul with lhsT = fused
    W = W_gcn@W_act giving out^T[a, d]; fused bias added via the scalar
    activation's per-partition bias; out^T accumulates in SBUF and is
    written once at the end (host transposes back).
"""

import os
import sys

for _p in ("/opt/trn_rl_repo",):
    if _p not in sys.path and os.path.isdir(_p):
        sys.path.insert(0, _p)

import numpy as np

# ---- problem constants (hardcoded per spec) ----
N, E, F, HID, A = 50000, 600000, 128, 128, 64
P = 128                      # partitions
NCORE = 8
DST_PER_CORE = 6250          # N / 8
NBLK = 49                    # ceil(6250/128) dst blocks per core
OUT_ROWS = NBLK * P          # 6272 padded out rows per core
HALF = 25152                 # nodes [0,HALF) in table A, [HALF,...) in table B
XROWS = 25216                # rows per half table (HALF + 64 zero pad rows)
ZROW_A = 25152               # a zero row in table A (explicit pad row)
ZROW_B = 25024               # zero row in table B (> N-HALF real rows)
MAXTPG = 8                   # hard cap: 1024 slots per gather call (ring cap)
CHUNK = 4                    # dst blocks per compute chunk
_CHUNKS = [(c * CHUNK, min(CHUNK, NBLK - c * CHUNK)) for c in range((NBLK + CHUNK - 1) // CHUNK)]

_prog_cache = {}


def _build_program(ntile, gmax):
    """Build the Bass program shared by all 8 cores.

    ntile: tuple of 98 ints — tiles (of 128 slots) per (block, half) group,
    group index g = blk*2 + half.
    gmax: tuple of 98 ints — cross-core max REAL slot count per group,
    rounded up to 16 (the rest of each gather call is -1 padding, which the
    SWDGE skips: no descriptors, no bytes)."""
    key = (tuple(ntile), tuple(gmax))
    if key in _prog_cache:
        return _prog_cache[key]

    import concourse.bacc as bacc
    import concourse.mybir as mybir
    import concourse.tile as tile
    from concourse.masks import make_identity

    f32 = mybir.dt.float32
    f16 = mybir.dt.float16
    i32 = mybir.dt.int32
    i16 = mybir.dt.int16
    Alu = mybir.AluOpType
    Act = mybir.ActivationFunctionType

    col_start = np.concatenate([[0], np.cumsum(ntile)]).astype(int)
    tot_col = int(col_start[-1])
    tot_slot = tot_col * P

    nc = bacc.Bacc(None, num_swdge_queues=4)

    xa = nc.dram_tensor("xa", [XROWS, F], f16, kind="ExternalInput")
    xb = nc.dram_tensor("xb", [XROWS, F], f16, kind="ExternalInput")
    xself = nc.dram_tensor("xself", [P, NBLK * F], f16, kind="ExternalInput")
    idxs = nc.dram_tensor("idxs", [P, tot_slot // 16], i16, kind="ExternalInput")
    dstloc = nc.dram_tensor("dstloc", [P, tot_col], f32, kind="ExternalInput")
    degprod = nc.dram_tensor("degprod", [P, tot_col], f32, kind="ExternalInput")
    degdst = nc.dram_tensor("degdst", [P, NBLK], f32, kind="ExternalInput")
    wgT = nc.dram_tensor("wgT", [HID, F], f32, kind="ExternalInput")
    wact = nc.dram_tensor("wact", [HID, A], f32, kind="ExternalInput")
    bgcn = nc.dram_tensor("bgcn", [HID, 1], f32, kind="ExternalInput")
    bact = nc.dram_tensor("bact", [A, 1], f32, kind="ExternalInput")
    out = nc.dram_tensor("out", [A, OUT_ROWS], f32, kind="ExternalOutput")

    with tile.TileContext(nc) as tc:
        with (
            tc.tile_pool(name="const", bufs=1) as cpool,
            tc.tile_pool(name="cpsum", bufs=1, space="PSUM") as cpsum,
            tc.tile_pool(name="xg", bufs=4) as xgpool,
            tc.tile_pool(name="sel", bufs=3) as spool,
            tc.tile_pool(name="acc", bufs=3, space="PSUM") as accpool,
            tc.tile_pool(name="outp", bufs=2, space="PSUM") as outppool,
            tc.tile_pool(name="flush", bufs=3) as fpool,
        ):
            # ---- constants / prologue ----
            idx_sb = cpool.tile([P, tot_slot // 16], i16)
            nc.sync.dma_start(out=idx_sb[:], in_=idxs[:])

            dstloc_sb = cpool.tile([P, tot_col], f32)
            nc.sync.dma_start(out=dstloc_sb[:], in_=dstloc[:])
            dstloc_h = cpool.tile([P, tot_col], f16)
            nc.vector.tensor_copy(out=dstloc_h[:], in_=dstloc_sb[:])

            # norm = rsqrt(deg[src]*deg[dst]) per slot (fp16 for 2x DVE mode)
            norm_sb = cpool.tile([P, tot_col], f32)
            nc.sync.dma_start(out=norm_sb[:], in_=degprod[:])
            nc.vector.reciprocal(out=norm_sb[:], in_=norm_sb[:])
            nc.scalar.activation(norm_sb[:], norm_sb[:], Act.Sqrt)
            norm_h = cpool.tile([P, tot_col], f16)
            nc.vector.tensor_copy(out=norm_h[:], in_=norm_sb[:])

            # dinv^2 = 1/deg for the self-loop diagonal
            dinvsq = cpool.tile([P, NBLK], f32)
            nc.sync.dma_start(out=dinvsq[:], in_=degdst[:])
            nc.vector.reciprocal(out=dinvsq[:], in_=dinvsq[:])

            ident = cpool.tile([P, P], f32)
            make_identity(nc, ident[:])
            ident_h = cpool.tile([P, P], f16)
            nc.vector.tensor_copy(out=ident_h[:], in_=ident[:])

            iota_i = cpool.tile([P, P], i32)
            nc.gpsimd.iota(iota_i[:], pattern=[[1, P]], base=0, channel_multiplier=0)
            iota_h = cpool.tile([P, P], f16)
            nc.vector.tensor_copy(out=iota_h[:], in_=iota_i[:])
            # iota materialized along the middle (dst) dim: iota_mid[p,d,c] = d
            chmax = max(int(col_start[(b0 + ncb) * 2] - col_start[b0 * 2])
                        for (b0, ncb) in _CHUNKS)
            iota_mid = cpool.tile([P, P, chmax], f16)
            nc.vector.tensor_copy(
                out=iota_mid[:],
                in_=iota_h[:].unsqueeze(2).broadcast_to([P, P, chmax]),
            )

            # per-block self-loop diagonal: diag[q, b, d] = (q==d) * dinvsq[q, b]
            diag = cpool.tile([P, NBLK, P], f16)
            nc.vector.tensor_tensor(
                out=diag[:],
                in0=ident_h[:].unsqueeze(1).broadcast_to([P, NBLK, P]),
                in1=dinvsq[:].unsqueeze(2).broadcast_to([P, NBLK, P]),
                op=Alu.mult,
            )

            xself_sb = cpool.tile([P, NBLK * F], f16)
            nc.sync.dma_start(out=xself_sb[:], in_=xself[:])

            wgT_sb = cpool.tile([HID, F], f32)
            nc.sync.dma_start(out=wgT_sb[:], in_=wgT[:])
            wact_sb = cpool.tile([HID, A], f32)
            nc.sync.dma_start(out=wact_sb[:], in_=wact[:])
            wf_ps = cpsum.tile([F, A], f32, space="PSUM", tag="cps")
            nc.tensor.matmul(wf_ps[:], lhsT=wgT_sb[:], rhs=wact_sb[:], start=True, stop=True)
            wf_sb = cpool.tile([F, A], f32)
            nc.vector.tensor_copy(out=wf_sb[:], in_=wf_ps[:])

            # fused bias, transposed: cbT[a] = sum_h bgcn[h]*W_act[h,a] + bact[a]
            bgcn_sb = cpool.tile([HID, 1], f32)
            nc.sync.dma_start(out=bgcn_sb[:], in_=bgcn[:])
            bact_sb = cpool.tile([A, 1], f32)
            nc.sync.dma_start(out=bact_sb[:], in_=bact[:])
            cb_ps = cpsum.tile([A, 1], f32, space="PSUM", tag="cps")
            nc.tensor.matmul(cb_ps[:], lhsT=wact_sb[:], rhs=bgcn_sb[:], start=True, stop=True)
            cb_sb = cpool.tile([A, 1], f32)
            nc.vector.tensor_copy(out=cb_sb[:], in_=cb_ps[:])
            nc.vector.tensor_tensor(out=cb_sb[:], in0=cb_sb[:], in1=bact_sb[:], op=Alu.add)

            # transposed output accumulator, written to DRAM once at the end
            out_all = cpool.tile([A, OUT_ROWS], f32)

            num_regs = {int(v): nc.gpsimd.to_reg(int(v))
                        for v in sorted(set(int(t) for t in gmax))}

            # ---- main loop over chunks of dst blocks ----
            qn = 0
            for ci, (b0, ncb) in enumerate(_CHUNKS):
                c0 = int(col_start[b0 * 2])
                ncols = int(col_start[(b0 + ncb) * 2] - c0)
                xg = xgpool.tile([P, ncols, F], f16, tag="xg")
                for i in range(ncb):
                    for h, tab in ((0, xa), (1, xb)):
                        g = (b0 + i) * 2 + h
                        nt = int(ntile[g])
                        num = nt * P
                        crel = int(col_start[g]) - c0
                        s0 = int(col_start[g]) * P
                        nc.gpsimd.dma_gather(
                            xg[:, crel: crel + nt, :],
                            tab[:],
                            idx_sb[:, s0 // 16: (s0 + num) // 16],
                            num,
                            num_regs[int(gmax[g])],
                            F,
                            single_packet=False,
                            queue_num=qn % 4,
                        )
                        qn += 1
                # norm-scaled one-hot, [P, dst, col] layout: every operand has
                # a stride-1 last dim -> DVE 2x perf mode on both big ops
                # sel[q, d, col] = (dstloc[q, col] == d) * norm[q, col]
                sel = spool.tile([P, P, ncols], f16, tag="sel")
                nc.vector.tensor_tensor(
                    out=sel[:],
                    in0=dstloc_h[:, c0:c0 + ncols].unsqueeze(1).broadcast_to([P, P, ncols]),
                    in1=iota_mid[:, :, :ncols],
                    op=Alu.is_equal,
                )
                nc.vector.tensor_tensor(
                    out=sel[:],
                    in0=sel[:],
                    in1=norm_h[:, c0:c0 + ncols].unsqueeze(1).broadcast_to([P, P, ncols]),
                    op=Alu.mult,
                )
                for i in range(ncb):
                    b = b0 + i
                    gA, gB = b * 2, b * 2 + 1
                    # accT[f, d] accumulated transposed: no flush transpose
                    acc = accpool.tile([P, P], f32, space="PSUM", tag="acc")
                    nc.tensor.matmul(
                        acc[:],
                        lhsT=xself_sb[:, b * F:(b + 1) * F],
                        rhs=diag[:, b, :],
                        start=True,
                        stop=False,
                    )
                    cols = list(range(int(col_start[gA]) - c0, int(col_start[gB + 1]) - c0))
                    for j, col in enumerate(cols):
                        nc.tensor.matmul(
                            acc[:],
                            lhsT=xg[:, col, :],
                            rhs=sel[:, :, col],
                            start=False,
                            stop=(j == len(cols) - 1),
                        )
                    # flush block b: out^T[a, d] = wf^T @ accT + cbT
                    accTs = fpool.tile([P, P], f32, tag="accTs")
                    nc.scalar.activation(accTs[:], acc[:], Act.Copy)
                    outp = outppool.tile([A, P], f32, space="PSUM", tag="outp")
                    nc.tensor.matmul(outp[:], lhsT=wf_sb[:], rhs=accTs[:], start=True, stop=True)
                    nc.scalar.activation(
                        out_all[:, b * P:(b + 1) * P], outp[:], Act.Identity,
                        bias=cb_sb[:, 0:1],
                    )
            nc.sync.dma_start(out=out[:], in_=out_all[:])

    nc.compile()
    _prog_cache[key] = nc
    return nc


def _preprocess(x, edge_index):
    """Host-side sharding: bucket edges by (core, dst block, src half) and
    build the static padded slot arrays. Integer/layout work only."""
    src = np.asarray(edge_index[0], dtype=np.int64)
    dst = np.asarray(edge_index[1], dtype=np.int64)

    in_deg = np.bincount(dst, minlength=N).astype(np.int64)
    deg_tot = in_deg + 1  # self loop

    core = dst // DST_PER_CORE
    loc = dst % DST_PER_CORE
    blk = loc >> 7
    dloc = loc & 127
    half = (src >= HALF).astype(np.int64)
    rowid = src - HALF * half
    dprod = deg_tot[src] * deg_tot[dst]

    # group = (core, blk, half); position within group via stable sort
    g = (core * NBLK + blk) * 2 + half
    order = np.argsort(g, kind="stable")
    g_sorted = g[order]
    cnt = np.bincount(g_sorted, minlength=NCORE * NBLK * 2)
    # static tile schedule: cross-core max per (blk, half) group
    cnt2 = cnt.reshape(NCORE, NBLK * 2)
    ntile = np.maximum(1, -(-cnt2.max(axis=0) // P))  # [98]
    if ntile.max() > MAXTPG:
        raise RuntimeError(f"group needs {ntile.max()} tiles > {MAXTPG}")
    col_start = np.concatenate([[0], np.cumsum(ntile)]).astype(np.int64)
    tot_col = int(col_start[-1])
    tot_slot = tot_col * P

    starts = np.zeros_like(cnt)
    starts[1:] = np.cumsum(cnt)[:-1]
    pos_in_group = np.arange(len(order)) - starts[g_sorted]

    blk_s = blk[order]
    half_s = half[order]
    g2 = blk_s * 2 + half_s
    col = col_start[g2] + (pos_in_group >> 7)
    p = pos_in_group & 127
    flat = col * P + p  # slot id within core

    core_s = core[order]
    rowid_s = rowid[order]
    dloc_s = dloc[order]
    dprod_s = dprod[order]

    # per-core output arrays (padded defaults; pad slots gather a zero row —
    # num_idxs_reg must equal the exact non-negative idx count, so variable
    # per-core counts would need per-core registers, which serialize the Q7s)
    idx_arr = np.empty((NCORE, tot_slot), dtype=np.int16)
    colg = np.repeat(np.arange(NBLK * 2), ntile)  # group of each column
    pad_idx = np.where(colg % 2 == 1, ZROW_B, ZROW_A).astype(np.int16)
    idx_arr[:] = np.repeat(pad_idx, P)[None, :]
    dst_arr = np.full((NCORE, tot_slot), -1.0, dtype=np.float32)
    dpr_arr = np.ones((NCORE, tot_slot), dtype=np.float32)
    gmax = (ntile * P).astype(np.int64)

    lin = core_s * tot_slot + flat
    idx_arr.reshape(-1)[lin] = rowid_s.astype(np.int16)
    dst_arr.reshape(-1)[lin] = dloc_s.astype(np.float32)
    dpr_arr.reshape(-1)[lin] = dprod_s.astype(np.float32)

    # idxs: 16-partition wrap replicated 8x -> [128, tot_slot//16]
    idx_wrap = idx_arr.reshape(NCORE, tot_slot // 16, 16).transpose(0, 2, 1)
    idx_rep = np.tile(idx_wrap, (1, 8, 1)).copy()



    # dstloc/degprod: [128, tot_col] with value at [p, col]
    dst_pc = dst_arr.reshape(NCORE, tot_col, P).transpose(0, 2, 1).copy()
    dpr_pc = dpr_arr.reshape(NCORE, tot_col, P).transpose(0, 2, 1).copy()

    # degdst: [NCORE, 128, NBLK]
    degdst = np.ones((NCORE, P, NBLK), dtype=np.float32)
    node = np.arange(N, dtype=np.int64)
    nc_ = node // DST_PER_CORE
    nl = node % DST_PER_CORE
    degdst[nc_, nl & 127, nl >> 7] = deg_tot.astype(np.float32)

    # x half tables (fp16, zero padded)
    x16 = x.astype(np.float16)
    xa = np.zeros((XROWS, F), dtype=np.float16)
    xa[:HALF] = x16[:HALF]
    xb = np.zeros((XROWS, F), dtype=np.float16)
    xb[: N - HALF] = x16[HALF:]

    # per-core own x rows, packed [128, NBLK*F]: partition p holds rows
    # {p, 128+p, ...} of the core's shard (for the self-loop diagonal matmul)
    xself = np.zeros((NCORE, P, NBLK * F), dtype=np.float16)
    for c in range(NCORE):
        shard = np.zeros((OUT_ROWS, F), dtype=np.float16)
        shard[:DST_PER_CORE] = x16[c * DST_PER_CORE:(c + 1) * DST_PER_CORE]
        xself[c] = shard.reshape(NBLK, P, F).transpose(1, 0, 2).reshape(P, NBLK * F)

    return ntile, gmax, xa, xb, xself, idx_rep, dst_pc, dpr_pc, degdst


def kernel(x, edge_index, W_gcn, b_gcn, W_act, b_act):
    from concourse.bass_utils import run_bass_kernel_spmd

    x = np.ascontiguousarray(np.asarray(x, dtype=np.float32))
    ntile, gmax, xa, xb, xself, idx_rep, dst_pc, dpr_pc, degdst = _preprocess(x, edge_index)

    wgT = np.ascontiguousarray(np.asarray(W_gcn, dtype=np.float32).T)
    wact = np.ascontiguousarray(np.asarray(W_act, dtype=np.float32))
    bg = np.ascontiguousarray(np.asarray(b_gcn, dtype=np.float32).reshape(HID, 1))
    ba = np.ascontiguousarray(np.asarray(b_act, dtype=np.float32).reshape(A, 1))

    nc = _build_program(tuple(int(v) for v in ntile), tuple(int(v) for v in gmax))
    in_maps = [
        {
            "xa": xa,
            "xb": xb,
            "xself": xself[c],
            "idxs": idx_rep[c],
            "dstloc": dst_pc[c],
            "degprod": dpr_pc[c],
            "degdst": degdst[c],
            "wgT": wgT,
            "wact": wact,
            "bgcn": bg,
            "bact": ba,
        }
        for c in range(NCORE)
    ]
    trace = bool(os.environ.get("GCN_TRACE"))
    res = run_bass_kernel_spmd(nc, in_maps, core_ids=list(range(NCORE)), trace=trace)
    kernel.last_results = res

    out = np.concatenate(
        [res.results[c]["out"].T[:DST_PER_CORE] for c in range(NCORE)], axis=0
    )
    return np.ascontiguousarray(out, dtype=np.float32)
